# revision 18
# baseline (speedup 1.0000x reference)
"""Two-layer modulated deformable conv (DCNv2) + sync-BN + ReLU for trn2.

Host does the cheap irregular work (offset conv, bilinear gather, BN); the
two big contractions (~39 GFLOP each) run on 8 NeuronCores, data-parallel
over (batch, HW-half). Per launch: ~7.5us runtime prologue + ~62us PE work
(147456 bf16 cycles/core @2.4GHz) + ~4.5us drain/store tail.

Layer 1 (C=256 -> O=128) runs in "z-form": bilinear sampling is linear and
positionwise, so out1[o,q] = sum_k,n coef[k,n,q] * z_k[o, pos(k,n,q)] with
z_k = W_k @ x a per-tap 1x1 conv. The device computes z from x directly
(4.8MB in / 14.7MB out per core) instead of streaming the 9x-redundant
im2col'd sampled tensor (37.7MB in); the host gathers z afterwards. To keep
the kernel PE- instead of DMA-bound (SDMA sustains ~330GB/s/core for loads+
stores combined), 4 of 9 taps store z as fp8 e3m4 (x4 scale folded into
those taps' weights, divided out of the gather coefs). Stores are split
across the gpsimd SWDGE queue and the sync HWDGE ring, byte-balanced, with
per-tap ring semaphores gating slab reuse.

Layer 2 (C=128 -> O=256) is an rhs-form GEMM over the host-sampled im2col
cols (z-form would store 37.7MB). The rhs HBM layout is tiled so every
k-group DMA is one fully contiguous block; the last of 3 k-groups is
host-quantized to e3m4 (x2 scale folded into its weight tiles), cutting the
load stream 18.9 -> 14.2MB/core. The first two superslabs load as per-k
slices so the PE streams while the 4-deep prefetch ring fills; output
blocks alternate between the gpsimd queue and inline stores on the ACT
drain ring (sem-gated: engine program order does NOT order async DGE reads
after a drain copy).

Both kernels issue junk matmuls on garbage SBUF at t=0 so the PE's HAM
activity window warms during the DMA fill (cold PE runs at 1.2GHz for the
first ~3.4us of activity). All weights are waited before the first real
matmul: the PE's LDWEIGHTS pull-ahead makes mid-stream weight gating racy.

End-to-end rel_fro 1.507e-2 (gate 2e-2): bf16 GEMM ~0.5e-2, L1 fp8 taps
~1.1e-2, L2 fp8 k-tiles ~0.9e-2, summed in quadrature (sim-verified).
"""

import os

import numpy as np
import ml_dtypes

BF16 = ml_dtypes.bfloat16
ML_E3M4 = ml_dtypes.float8_e3m4

B, CIN, H, W = 4, 256, 128, 128
MID, COUT = 128, 256
HW = H * W
K2 = 9
_EPS = 1e-5

_KY = np.array([-1, -1, -1, 0, 0, 0, 1, 1, 1], dtype=np.float32)
_KX = np.array([-1, 0, 1, -1, 0, 1, -1, 0, 1], dtype=np.float32)


# ---------------------------------------------------------------- host pieces
def _im2col(x):
    """x [B,C,H,W] -> cols [B, C*9, H*W] (3x3 SAME, zero pad)."""
    b, c, h, w = x.shape
    xp = np.zeros((b, c, h + 2, w + 2), dtype=x.dtype)
    xp[:, :, 1:-1, 1:-1] = x
    cols = np.empty((b, c, 9, h, w), dtype=x.dtype)
    k = 0
    for dy in range(3):
        for dx in range(3):
            cols[:, :, k] = xp[:, :, dy:dy + h, dx:dx + w]
            k += 1
    return cols.reshape(b, c * 9, h * w)


def _conv3x3_host(cols, w, bias):
    """cols [B, C*9, HW], w [O,C,3,3] -> [B, O, HW]."""
    o = w.shape[0]
    wr = w.reshape(o, -1)
    out = np.matmul(wr[None], cols)  # [B, O, HW]
    return out + bias[None, :, None]


def _offsets_for_layer(x, w_off, b_off):
    """x [B,C,H,W] -> bilinear corner indices+coefs per (b, tap, pixel).

    Returns idx [B,9,4,HW] int32 (flat pixel index, clipped) and
    coef [B,9,4,HW] float32 (bilinear weight x validity x sigmoid mask).
    """
    b, c, h, w = x.shape
    om = _conv3x3_host(_im2col(x), w_off, b_off).reshape(b, 27, h, w)
    off_y = om[:, :K2]
    off_x = om[:, K2:2 * K2]
    mask = 1.0 / (1.0 + np.exp(-om[:, 2 * K2:]))
    yy = np.arange(h, dtype=np.float32)
    xx = np.arange(w, dtype=np.float32)
    py = yy[None, None, :, None] + _KY[None, :, None, None] + off_y  # [B,9,H,W]
    px = xx[None, None, None, :] + _KX[None, :, None, None] + off_x

    y0 = np.floor(py)
    x0 = np.floor(px)
    ly = py - y0
    lx = px - x0
    y0i = y0.astype(np.int32)
    x0i = x0.astype(np.int32)

    idx = np.empty((b, K2, 4, h * w), dtype=np.int32)
    coef = np.empty((b, K2, 4, h * w), dtype=np.float32)
    corners = ((y0i, x0i, (1 - ly) * (1 - lx)),
               (y0i, x0i + 1, (1 - ly) * lx),
               (y0i + 1, x0i, ly * (1 - lx)),
               (y0i + 1, x0i + 1, ly * lx))
    for n, (yi, xi, cf) in enumerate(corners):
        valid = ((yi >= 0) & (yi < h) & (xi >= 0) & (xi < w)).astype(np.float32)
        fi = np.clip(yi, 0, h - 1) * w + np.clip(xi, 0, w - 1)
        idx[:, :, n] = fi.reshape(b, K2, h * w)
        coef[:, :, n] = (cf * valid * mask).reshape(b, K2, h * w)
    return idx, coef


def _gather_combine(z, idx, coef):
    """z [B,9,O,HW], idx/coef [B,9,4,HW] -> y [B,O,HW] = sum_k,n coef*z[...,idx]."""
    b, k2, o, hw = z.shape
    y = np.zeros((b, o, hw), dtype=np.float32)
    for bi in range(b):
        for k in range(k2):
            zk = z[bi, k]
            for n in range(4):
                y[bi] += zk[:, idx[bi, k, n]] * coef[bi, k, n][None]
    return y


def _bilinear_modulated(x, py, px, mask):
    """x [C,H,W]; py,px,mask [9,H,W] -> modulated samples [C*9, HW]."""
    c, h, w = x.shape
    y0 = np.floor(py)
    x0 = np.floor(px)
    ly = py - y0
    lx = px - x0
    y0i = y0.astype(np.int32)
    x0i = x0.astype(np.int32)
    flat = x.reshape(c, h * w)

    def gather(yi, xi):
        valid = ((yi >= 0) & (yi < h) & (xi >= 0) & (xi < w)).astype(np.float32)
        idx = np.clip(yi, 0, h - 1) * w + np.clip(xi, 0, w - 1)
        v = flat[:, idx.reshape(-1)].reshape(c, *yi.shape)
        return v * valid[None]

    v00 = gather(y0i, x0i)
    v01 = gather(y0i, x0i + 1)
    v10 = gather(y0i + 1, x0i)
    v11 = gather(y0i + 1, x0i + 1)
    w00 = ((1 - ly) * (1 - lx) * mask)[None]
    w01 = ((1 - ly) * lx * mask)[None]
    w10 = (ly * (1 - lx) * mask)[None]
    w11 = (ly * lx * mask)[None]
    s = v00 * w00 + v01 * w01 + v10 * w10 + v11 * w11  # [C,9,H,W]
    return s.reshape(c * 9, h * w).astype(np.float32)


def _sampled_for_layer(x, w_off, b_off):
    """x [B,C,H,W] -> modulated sampled cols [B, C*9, HW]."""
    b, c, h, w = x.shape
    om = _conv3x3_host(_im2col(x), w_off, b_off).reshape(b, 27, h, w)
    off_y = om[:, :K2]
    off_x = om[:, K2:2 * K2]
    mask = 1.0 / (1.0 + np.exp(-om[:, 2 * K2:]))
    yy = np.arange(h, dtype=np.float32)
    xx = np.arange(w, dtype=np.float32)
    py = yy[None, None, :, None] + _KY[None, :, None, None] + off_y  # [B,9,H,W]
    px = xx[None, None, None, :] + _KX[None, :, None, None] + off_x
    out = np.empty((b, c * 9, h * w), dtype=np.float32)
    for i in range(b):
        out[i] = _bilinear_modulated(x[i], py[i], px[i], mask[i])
    return out


def _bn_relu(x, gamma, beta):
    """x [B,O,HW] -> same, sync-BN (biased var) + affine + relu."""
    mu = x.mean(axis=(0, 2), keepdims=True)
    var = ((x - mu) ** 2).mean(axis=(0, 2), keepdims=True)
    y = (x - mu) / np.sqrt(var + _EPS)
    y = y * gamma[None, :, None] + beta[None, :, None]
    return np.maximum(y, 0.0)


# ------------------------------------------------------------ L1 bass kernel
# z-form: per (batch, hw-half) core, compute z[tap][o, q] = sum_c W1[o,c,tap]
# * x[c, q] for 9 taps, O=128, C=256 (2 c-tiles), 8192 columns.
# Pipelined over 4 column panels of 2048 (4 psum chunks of 512 each).
_L1_T = 9          # taps
_L1_CT = 2         # c-tiles (contraction 256)
_L1_NP = 4         # column panels
_L1_PW = 2048      # panel width
_L1_CH = 4         # 512-wide psum chunks per panel
_L1_WARM = 9       # junk warm-up matmuls
# z for these taps is stored as fp8 e3m4 (x4 scale folded into the weights,
# divided back out in the host gather): halves those taps' store bytes, which
# takes the kernel from DMA-bound to PE-bound. 4/9 taps keeps the end-to-end
# rel_fro at 1.2e-2 (sim) vs the 2e-2 gate.
_L1_TAPS8 = (1, 3, 5, 7)
_L1_ZSCALE = 4.0
# store-queue assignment, byte-balanced: gpsimd {0,1,2,4}, sync {3,5,6,7,8}
_L1_QG = (0, 1, 2, 4)


def _build_l1_nc():
    from contextlib import ExitStack

    import concourse.bass as bass
    import concourse.mybir as mybir

    f32 = mybir.dt.float32
    bf16 = mybir.dt.bfloat16
    T, CT, NP, PW, CH = _L1_T, _L1_CT, _L1_NP, _L1_PW, _L1_CH
    NCHUNK = NP * T * CH           # 144 total psum chunks
    PB = T * CH                    # 36 banks (chunks) per panel
    T8 = _L1_TAPS8                 # e3m4-stored taps
    TB = tuple(k for k in range(T) if k not in T8)
    bpos = {k: i for i, k in enumerate(TB)}   # tap -> bf16 slab index
    epos = {k: i for i, k in enumerate(T8)}   # tap -> e3m4 slab index

    fp8 = mybir.dt.float8e3
    nc = bass.Bass()
    xin = nc.dram_tensor("xin", [NP * CT * 128, PW], bf16, kind="ExternalInput")
    wt = nc.dram_tensor("wt", [128, T * CT * 128], bf16, kind="ExternalInput")
    zout = nc.dram_tensor("zout", [NP * len(TB) * 128, PW], bf16,
                          kind="ExternalOutput")
    zout8 = nc.dram_tensor("zout8", [NP * len(T8) * 128, PW], fp8,
                           kind="ExternalOutput")

    with ExitStack() as es:
        wtile = es.enter_context(nc.sbuf_tensor("wtile", [128, T * CT * 128], bf16))
        xbs = [[es.enter_context(nc.sbuf_tensor(f"xb{i}_{c}", [128, PW], bf16))
                for c in range(CT)] for i in range(2)]
        zbs = [es.enter_context(nc.sbuf_tensor(f"zb{i}", [128, len(TB) * PW],
                                               bf16)) for i in range(2)]
        zb8s = [es.enter_context(nc.sbuf_tensor(f"zb8{i}", [128, len(T8) * PW],
                                                fp8)) for i in range(2)]
        psums = [es.enter_context(nc.psum_tensor(f"psum{i}", [128, 512], f32))
                 for i in range(8)]
        wts = [es.enter_context(nc.semaphore(name=f"wts{i}")) for i in range(2)]
        # panel-0 512-col slice sems (priority fill) + rings for panels 1..3
        xhs = [[es.enter_context(nc.semaphore(name=f"xh{c}_{h}"))
                for h in range(CH)] for c in range(CT)]
        xls = [[es.enter_context(nc.semaphore(name=f"xl{i}_{c}"))
                for c in range(CT)] for i in range(2)]
        pbank = es.enter_context(nc.semaphore(name="pbank"))
        dve = es.enter_context(nc.semaphore(name="dve"))
        dva = es.enter_context(nc.semaphore(name="dva"))
        # store-completion rings: even taps store via gpsimd (stg), odd taps
        # via the sync HWDGE ring (sty) so the end-of-kernel tail halves
        stg = [es.enter_context(nc.semaphore(name=f"stg{k}")) for k in range(T)]
        sty = [es.enter_context(nc.semaphore(name=f"sty{k}")) for k in range(T)]
        block = es.enter_context(nc.Block())

        def st_sem(k):
            return stg[k] if k in _L1_QG else sty[k]

        def zb_dst(p, k):
            if k in T8:
                return zb8s[p % 2][:, epos[k] * PW:(epos[k] + 1) * PW]
            return zbs[p % 2][:, bpos[k] * PW:(bpos[k] + 1) * PW]

        def zout_dst(p, k):
            if k in T8:
                i = p * len(T8) + epos[k]
                return zout8[i * 128:(i + 1) * 128, :]
            i = p * len(TB) + bpos[k]
            return zout[i * 128:(i + 1) * 128, :]

        def xin_blk(p, c):
            return xin[(p * CT + c) * 128:(p * CT + c + 1) * 128, :]

        # drain bookkeeping: dve counts even-g drains in g order, dva odd
        def drain(eng, g, sem, isvec):
            p, r = g // PB, g % PB
            k, ch = r // CH, r % CH
            eng.wait_ge(pbank, g + 1)
            if p >= 2 and ch == (0 if isvec else 1):
                # zb slot reused; store of (p-2, k) must be done
                eng.wait_ge(st_sem(k), 16 * (p - 1))
            dst = zb_dst(p, k)[:, ch * 512:(ch + 1) * 512]
            src = psums[g % 8][:]
            if hasattr(eng, "tensor_copy"):
                eng.tensor_copy(dst, src).then_inc(sem, 1)
            else:
                eng.copy(dst, src).then_inc(sem, 1)

        def store(eng, p, k):
            g_hi = p * PB + k * CH + CH - 1     # last (odd) chunk of (p,k)
            eng.wait_ge(dve, (g_hi - 1) // 2 + 1)
            eng.wait_ge(dva, (g_hi - 1) // 2 + 1)
            eng.dma_start(zout_dst(p, k), zb_dst(p, k)).then_inc(st_sem(k), 16)

        WH = T * CT * 128 // 2

        @block.sync
        def _(sync):
            # weight low half first (split across both HWDGE rings so the
            # full weight lands ~1us sooner; all of it is waited before the
            # first real matmul -- mid-stream weight gating is racy), then
            # panel-0 c0 in 512-col slices, then panel-1 c0
            sync.dma_start(xbs[0][0][:, 0:512],
                           xin_blk(0, 0)[:, 0:512]).then_inc(xhs[0][0], 16)
            sync.dma_start(wtile[:, 0:WH], wt[:, 0:WH]).then_inc(wts[0], 16)
            for h in range(1, CH):
                sync.dma_start(xbs[0][0][:, h * 512:(h + 1) * 512],
                               xin_blk(0, 0)[:, h * 512:(h + 1) * 512]
                               ).then_inc(xhs[0][h], 16)
            sync.dma_start(xbs[1][0][:], xin_blk(1, 0)).then_inc(xls[1][0], 16)
            sync.wait_ge(pbank, PB)
            sync.dma_start(xbs[0][0][:], xin_blk(2, 0)).then_inc(xls[0][0], 16)
            for k in sorted((k for k in range(T) if k not in _L1_QG),
                            key=lambda k: (k in T8, k)):
                store(sync, 0, k)
            sync.wait_ge(pbank, 2 * PB)
            sync.dma_start(xbs[1][0][:], xin_blk(3, 0)).then_inc(xls[1][0], 16)
            for p in range(1, NP):
                for k in sorted((k for k in range(T) if k not in _L1_QG),
                                key=lambda k: (k in T8, k)):
                    store(sync, p, k)
            for k in range(T):
                if k not in _L1_QG:
                    sync.wait_ge(sty[k], 16 * NP)

        @block.scalar
        def _(scalar):
            # weight high half, then c1 loads between odd-chunk drains
            scalar.dma_start(xbs[0][1][:, 0:512],
                             xin_blk(0, 1)[:, 0:512]).then_inc(xhs[1][0], 16)
            scalar.dma_start(wtile[:, WH:], wt[:, WH:]).then_inc(wts[1], 16)
            for h in range(1, CH):
                scalar.dma_start(xbs[0][1][:, h * 512:(h + 1) * 512],
                                 xin_blk(0, 1)[:, h * 512:(h + 1) * 512]
                                 ).then_inc(xhs[1][h], 16)
            scalar.dma_start(xbs[1][1][:], xin_blk(1, 1)).then_inc(xls[1][1], 16)
            for p in range(NP):
                for r in range(1, PB, 2):
                    drain(scalar, p * PB + r, dva, False)
                if p + 2 < NP:
                    q = p + 2
                    scalar.wait_ge(pbank, PB * (q - 1))
                    scalar.dma_start(xbs[q % 2][1][:], xin_blk(q, 1)
                                     ).then_inc(xls[q % 2][1], 16)

        @block.tensor
        def _(tensor):
            # warm the HAM activity window on garbage while DMAs fill
            for j in range(_L1_WARM):
                tensor.matmul(psums[j % 8][:], wtile[:, 0:128],
                              xbs[0][0][:, 0:512], start=True, stop=True)
            tensor.wait_ge(wts[0], 16)
            tensor.wait_ge(wts[1], 16)
            for g in range(NCHUNK):
                p, r = g // PB, g % PB
                k, ch = r // CH, r % CH
                if p == 0:
                    if k == 0:
                        for c in range(CT):
                            tensor.wait_ge(xhs[c][ch], 16)
                elif r == 0:
                    for c in range(CT):
                        tensor.wait_ge(xls[p % 2][c], 16 * ((p + 1) // 2))
                if g >= 8:
                    op = g - 8
                    tensor.wait_ge(dve if op % 2 == 0 else dva, op // 2 + 1)
                tensor.matmul(
                    psums[g % 8][:],
                    wtile[:, (k * CT) * 128:(k * CT + 1) * 128],
                    xbs[p % 2][0][:, ch * 512:(ch + 1) * 512],
                    start=True, stop=False)
                tensor.matmul(
                    psums[g % 8][:],
                    wtile[:, (k * CT + 1) * 128:(k * CT + 2) * 128],
                    xbs[p % 2][1][:, ch * 512:(ch + 1) * 512],
                    start=False, stop=True).then_inc(pbank, 1)

        @block.vector
        def _(vector):
            for g in range(0, NCHUNK, 2):
                drain(vector, g, dve, True)

        @block.gpsimd
        def _(gpsimd):
            for p in range(NP):
                for k in sorted(_L1_QG, key=lambda k: (k in T8, k)):
                    store(gpsimd, p, k)
            for k in _L1_QG:
                gpsimd.wait_ge(stg[k], 16 * NP)
    return nc


def _pack_l1_x(x_half):
    """x_half [256, 8192] fp32 -> [NP*CT*128, PW] bf16, block (p,c) rows."""
    t = x_half.reshape(_L1_CT, 128, _L1_NP, _L1_PW).transpose(2, 0, 1, 3)
    return np.ascontiguousarray(t.astype(BF16)).reshape(_L1_NP * _L1_CT * 128,
                                                        _L1_PW)


def _pack_l1_w(w1):
    """w1 [128, 256, 3, 3] -> wt [128, T*CT*128] bf16.

    wt[p, (k*CT+c)*128 + o] = w1[o, c*128+p, k]; e3m4-stored taps are
    pre-scaled by _L1_ZSCALE (divided back out in the gather coefs).
    """
    a = w1.reshape(128, _L1_CT, 128, K2)       # [o, c, p, k]
    a = np.ascontiguousarray(a.transpose(2, 3, 1, 0)).copy()  # [p, k, c, o]
    for k in _L1_TAPS8:
        a[:, k] *= _L1_ZSCALE
    return np.ascontiguousarray(a.astype(BF16)).reshape(128, _L1_T * _L1_CT * 128)


def _unpack_l1_z(blocks, blocks8):
    """per-core z blocks (bf16 taps + scaled e3m4 taps) -> [B, 9, 128, HW]."""
    TB = [k for k in range(_L1_T) if k not in _L1_TAPS8]
    zb = blocks.astype(np.float32).reshape(B, 2, _L1_NP, len(TB), 128, _L1_PW)
    z8 = blocks8.astype(np.float32).reshape(B, 2, _L1_NP, len(_L1_TAPS8),
                                            128, _L1_PW) / _L1_ZSCALE
    z = np.empty((B, _L1_T, 128, 2, _L1_NP, _L1_PW), dtype=np.float32)
    for i, k in enumerate(TB):
        z[:, k] = zb[:, :, :, i].transpose(0, 3, 1, 2, 4)
    for i, k in enumerate(_L1_TAPS8):
        z[:, k] = z8[:, :, :, i].transpose(0, 3, 1, 2, 4)
    return np.ascontiguousarray(z).reshape(B, _L1_T, 128, HW)


def _device_l1(x, w1):
    """x [B,256,H,W], w1 [128,256,3,3] -> z [B,9,128,HW] fp32 on 8 cores."""
    import time

    from concourse import bass_utils

    key = ("l1",)
    if key not in _NC_CACHE:
        _NC_CACHE[key] = _build_l1_nc()
    nc = _NC_CACHE[key]

    wt = _pack_l1_w(w1)
    xf = x.reshape(B, CIN, HW)
    in_maps = []
    for s in range(8):
        b, hh = s // 2, s % 2
        xh = xf[b, :, hh * (HW // 2):(hh + 1) * (HW // 2)]
        in_maps.append({"xin": _pack_l1_x(xh), "wt": wt})

    kwargs = {}
    base = os.environ.get("BASS_KERNEL_TMPDIR")
    if base:
        d = os.path.join(base, f"p{os.getpid()}_call{len(DEVICE_STATS)}")
        os.makedirs(d, exist_ok=True)
        kwargs["tmpdir"] = d

    t0 = time.perf_counter_ns()
    res = bass_utils.run_bass_kernel_spmd(nc, in_maps, core_ids=list(range(8)),
                                          **kwargs)
    t1 = time.perf_counter_ns()
    DEVICE_STATS.append({"wall_ns": t1 - t0,
                         "exec_time_ns": res.exec_time_ns})
    blocks = np.stack([np.asarray(res.results[s]["zout"]) for s in range(8)])
    blocks8 = np.stack([np.asarray(res.results[s]["zout8"]).view(ML_E3M4)
                        for s in range(8)])
    return _unpack_l1_z(blocks, blocks8)


def _l1_forward(x, w1):
    try:
        return _device_l1(x, w1)
    except Exception as e:  # pragma: no cover - device fallback
        import traceback
        traceback.print_exc()
        print(f"[kernel] L1 device path failed ({e!r}); numpy fallback")
        wr = w1.reshape(128, CIN, K2)          # [o, c, k]
        xf = x.reshape(B, CIN, HW)
        z = np.einsum('ock,bcq->bkoq', wr.astype(np.float32),
                      xf.astype(np.float32))
        return np.ascontiguousarray(z)


# ------------------------------------------------------------ L2 bass kernel
_NT = 512        # PSUM-bank-sized matmul moving width (512 fp32 accum cols)
_NJ = 4          # psum banks per superslab (2 sets of 4 ping-pong)
_NRB = 6         # rhs superslab ring depth
_NOB = 8         # output staging buffers
_L2_WARM = 8     # junk warm-up matmuls
# trailing k-groups of the rhs stream quantized to e3m4 on host (x2 scale
# folded into those k-tiles' weights): cuts the 18.9MB/core load stream to
# 14.2MB so the kernel is PE- not DMA-bound. 3 of 9 k-tiles adds ~0.9e-2
# rel_fro (sim: total 1.49e-2 vs the 2e-2 gate).
_L2_F8G = 1      # of G=3 groups
_L2_SSCALE = 2.0


def _build_matmul_nc(nk, odim, ns, kq, nf8=0):
    """out = lhsT.T @ rhs (bf16 in, fp32 PSUM, bf16 out), pre-tiled layouts.

    nm = odim/128 m-tiles; nch = 4/nm column chunks of 512; superslab width
    WS = nch*512; ns superslabs of ncols = ns*WS per core.

    HBM layouts (host pre-arranges):
      rhs [ns*G*128, kq*WS]    k-group (S,g) rows (S*G+g)*128.. : one fully
                               contiguous block per group DMA
      wt  [128, nk*odim]       wt[p, k*odim+o] = lhsT[k*128+p, o]
      out [ns*4*128, _NT]      block o = S*4 + m*nch + c

    PE loop is k-outer within a superslab: per k it loads one 128x128 weight
    tile and streams nm*nch matmuls into 4 PSUM banks, accumulating over all
    nk. Superslab S uses PSUM bank set S%2 (ping-pong) so a set drains on
    DVE/ACT during the entire next superslab. rhs streams in kq-sized
    k-groups on a ring _NRB superslabs deep.

    DMA-completion semaphores are RINGS (one sem per buffer slot): every
    ring sem has its next DMA issued only after the wait for the previous
    one passed, so every wait is exact.
    """
    from contextlib import ExitStack

    import concourse.bass as bass
    import concourse.mybir as mybir

    f32 = mybir.dt.float32
    bf16 = mybir.dt.bfloat16
    nm = odim // 128
    nch = _NJ // nm              # column chunks per superslab
    WS = nch * _NT               # superslab width
    G = nk // kq                 # k load groups per superslab
    assert G * kq == nk
    GB = G - nf8                 # leading bf16 groups
    kb = GB * kq                 # first fp8 k-tile
    NO = ns * _NJ                # output blocks
    nrb = min(_NRB, ns)
    fp8 = mybir.dt.float8e3
    nc = bass.Bass()
    rhs = nc.dram_tensor("rhs", [ns * GB * 128, kq * WS], bf16,
                         kind="ExternalInput")
    if nf8:
        rhs8 = nc.dram_tensor("rhs8", [ns * nf8 * 128, kq * WS], fp8,
                              kind="ExternalInput")
    wt = nc.dram_tensor("wt", [128, nk * odim], bf16, kind="ExternalInput")
    out = nc.dram_tensor("out", [NO * 128, _NT], bf16, kind="ExternalOutput")

    with ExitStack() as es:
        wtile = es.enter_context(nc.sbuf_tensor("wtile", [128, nk * odim], bf16))
        rbufs = [es.enter_context(nc.sbuf_tensor(f"rbuf{i}", [128, kb * WS], bf16))
                 for i in range(nrb)]
        rb8s = [es.enter_context(nc.sbuf_tensor(f"rb8_{i}",
                                                [128, nf8 * kq * WS], fp8))
                for i in range(nrb)] if nf8 else None
        obufs = [es.enter_context(nc.sbuf_tensor(f"obuf{i}", [128, _NT], bf16))
                 for i in range(_NOB)]
        psums = [es.enter_context(nc.psum_tensor(f"psum{i}", [128, _NT], f32))
                 for i in range(2 * _NJ)]
        wts = [es.enter_context(nc.semaphore(name=f"wt{g}")) for g in range(G)]
        lds = [[es.enter_context(nc.semaphore(name=f"ld{i}_{g}"))
                for g in range(G)] for i in range(nrb)]
        # single-k slices of the first two superslabs, so the PE starts
        # early and streams at fine granularity while the ring fills
        lks = [[es.enter_context(nc.semaphore(name=f"l{S}k{kk}"))
                for kk in range(nk)] for S in range(min(2, ns))]
        peg = es.enter_context(nc.semaphore(name="peg"))
        pbank = es.enter_context(nc.semaphore(name="pbank"))
        dve = es.enter_context(nc.semaphore(name="dve"))
        dva = es.enter_context(nc.semaphore(name="dva"))
        sts = [es.enter_context(nc.semaphore(name=f"st{i}")) for i in range(_NOB)]
        block = es.enter_context(nc.Block())

        def rh_region(S, g):
            if g >= GB:
                i = S * nf8 + g - GB
                return rhs8[i * 128:(i + 1) * 128, :]
            return rhs[(S * GB + g) * 128:(S * GB + g + 1) * 128, :]

        def rb_region(S, g):
            if g >= GB:
                i = g - GB
                return rb8s[S % nrb][:, i * kq * WS:(i + 1) * kq * WS]
            return rbufs[S % nrb][:, g * kq * WS:(g + 1) * kq * WS]

        def rb_k(S, k):
            # SBUF slab slice holding k-tile k of superslab S
            if k >= kb:
                return rb8s[S % nrb][:, (k - kb) * WS:(k - kb + 1) * WS]
            return rbufs[S % nrb][:, k * WS:(k + 1) * WS]

        dual = G > 3  # two HWDGE load rings only pay off for a big stream

        def lds_cnt(S):
            # how many times lds[S % nrb][g] has been inc'd up to and
            # including superslab S (superslabs 0,1 use the lks slice sems)
            return len([s for s in range(2, S + 1) if s % nrb == S % nrb])

        def load_stream(eng, parity):
            # rhs k-group loads with this (S*G+g) parity, gated on PE progress.
            # Weight piece g rides just before rhs(0, g) on the same ring so
            # both rings deliver strictly in the order the PE consumes.
            for S in range(ns):
                for g in range(G):
                    if ((S * G + g) % 2 if dual else 0) != parity:
                        continue
                    if S == 0:
                        eng.dma_start(wtile[:, g * kq * odim:(g + 1) * kq * odim],
                                      wt[:, g * kq * odim:(g + 1) * kq * odim]
                                      ).then_inc(wts[g], 16)
                    if S < 2:
                        # per-k slices: fine-grained pipe while the ring fills
                        for kk in range(kq):
                            k = g * kq + kk
                            eng.dma_start(
                                rb_k(S, k),
                                rh_region(S, g)[:, kk * WS:(kk + 1) * WS]
                            ).then_inc(lks[S][k], 16)
                        continue
                    if S >= nrb:
                        # rbuf region (S-nrb, g) must be consumed by PE first
                        if g < G - 1:
                            eng.wait_ge(peg, (S - nrb) * (G - 1) + g + 1)
                        else:
                            eng.wait_ge(pbank, (S - nrb + 1) * _NJ)
                    eng.dma_start(rb_region(S, g), rh_region(S, g)
                                  ).then_inc(lds[S % nrb][g], 16)

        def drain(eng, o, sem):
            # PSUM block o -> bf16 obuf on this engine's ALU
            S, j = o // _NJ, o % _NJ
            eng.wait_ge(pbank, o + 1)
            if o >= _NOB:
                # obuf slot reused from store o - _NOB on this ring sem
                eng.wait_ge(sts[o % _NOB], 16 * ((o - _NOB) // _NOB + 1))
            src = psums[(S % 2) * _NJ + j][:]
            dst = obufs[o % _NOB][:]
            if hasattr(eng, "tensor_copy"):
                eng.tensor_copy(dst, src).then_inc(sem, 1)
            else:
                eng.copy(dst, src).then_inc(sem, 1)  # ACT activation-copy

        @block.sync
        def _(sync):
            # even-parity k-group loads on the SP HWDGE ring
            load_stream(sync, 0)
            for j in range(_NOB):
                nstores = (NO - j + _NOB - 1) // _NOB
                if nstores:
                    sync.wait_ge(sts[j], 16 * nstores)

        @block.scalar
        def _(scalar):
            # odd-parity k-group loads on the ACT HWDGE ring (dual), else the
            # ACT ALU drains odd PSUM blocks (and stores them inline on this
            # ring, halving the gpsimd store queue + the end-of-kernel tail)
            load_stream(scalar, 1)
            if not dual:
                for o in range(1, NO, 2):
                    drain(scalar, o, dva)
                    # the copy's @complete inc must fire before the DGE reads
                    # obuf (engine program order does NOT cover async DMA reads)
                    scalar.wait_ge(dva, o // 2 + 1)
                    scalar.dma_start(out[o * 128:(o + 1) * 128, :],
                                     obufs[o % _NOB][:]).then_inc(sts[o % _NOB], 16)

        @block.tensor
        def _(tensor):
            for j in range(_L2_WARM):
                tensor.matmul(psums[j % (2 * _NJ)][:], wtile[:, 0:128],
                              rbufs[0][:, 0:_NT], start=True, stop=True)
            for S in range(ns):
                pp = (S % 2) * _NJ   # psum set for this superslab
                for g in range(G):
                    if S == 0:
                        tensor.wait_ge(wts[g], 16)
                    if S >= 2:
                        tensor.wait_ge(lds[S % nrb][g], 16 * lds_cnt(S))
                    mm = None
                    for kk in range(kq):
                        k = g * kq + kk
                        if S < 2:
                            tensor.wait_ge(lks[S][k], 16)
                        for m in range(nm):
                            for c in range(nch):
                                j = m * nch + c
                                if k == 0 and S >= 2:
                                    # bank reused from superslab S-2; drained?
                                    op = (S - 2) * _NJ + j
                                    if dual:
                                        tensor.wait_ge(dve, op + 1)
                                    else:
                                        tensor.wait_ge(
                                            dve if op % 2 == 0 else dva,
                                            op // 2 + 1)
                                mm = tensor.matmul(
                                    psums[pp + j][:],
                                    wtile[:, k * odim + m * 128:
                                          k * odim + (m + 1) * 128],
                                    rb_k(S, k)[:, c * _NT:(c + 1) * _NT],
                                    start=(k == 0), stop=(k == nk - 1))
                                if k == nk - 1:
                                    mm.then_inc(pbank, 1)
                    if g < G - 1:
                        mm.then_inc(peg, 1)

        @block.vector
        def _(vector):
            for o in (range(NO) if dual else range(0, NO, 2)):
                drain(vector, o, dve)

        @block.gpsimd
        def _(gpsimd):
            # even blocks stored on SWDGE (odd ones ride the ACT ring)
            for o in (range(NO) if dual else range(0, NO, 2)):
                gpsimd.wait_ge(dve, o + 1 if dual else o // 2 + 1)
                gpsimd.dma_start(out[o * 128:(o + 1) * 128, :],
                                 obufs[o % _NOB][:]).then_inc(sts[o % _NOB], 16)
    return nc


_NC_CACHE = {}
DEVICE_STATS = []  # one entry per device invocation: {wall_ns, exec_time_ns}


def _tile_rhs(sampled, nk, ns, ws, kq, nf8=0, scale=1.0):
    """sampled [B, K, HW] fp32 -> (bf16 blocks, e3m4 blocks) per group."""
    bdim = sampled.shape[0]
    g = nk // kq
    gb = g - nf8
    t = sampled.reshape(bdim, g, kq, 128, 2, ns, ws)
    t = t.transpose(0, 4, 5, 1, 3, 2, 6)  # [b, hh, S, g, p, kk, w]
    bf = np.ascontiguousarray(t[:, :, :, :gb].astype(BF16)).reshape(
        bdim, 2, ns * gb * 128, kq * ws)
    f8 = None
    if nf8:
        q = np.clip(t[:, :, :, gb:] * scale, -15.5, 15.5)
        f8 = np.ascontiguousarray(q.astype(ML_E3M4)).reshape(
            bdim, 2, ns * nf8 * 128, kq * ws)
    return bf, f8


def _untile_out(blocks, nm, ns, nch):
    """[8, ns*nm*nch*128, _NT] bf16 -> [B, nm*128, HW] fp32."""
    t = blocks.astype(np.float32).reshape(B, 2, ns, nm, nch, 128, _NT)
    t = t.transpose(0, 3, 5, 1, 2, 4, 6)  # [b, m, p, hh, S, c, w]
    return np.ascontiguousarray(t).reshape(B, nm * 128, HW)


def _device_contract(sampled, wr):
    """sampled [B, K, HW], wr [O, K] -> [B, O, HW] on 8 cores (b, hw-half)."""
    import time

    from concourse import bass_utils

    bdim, kdim, hw = sampled.shape
    odim = wr.shape[0]
    half = hw // 2
    nk = kdim // 128
    nm = odim // 128
    nch = _NJ // nm
    ws = nch * _NT
    ns = half // ws
    kq = 3
    nf8 = _L2_F8G
    key = (nk, odim, ns, kq, nf8)
    if key not in _NC_CACHE:
        _NC_CACHE[key] = _build_matmul_nc(nk, odim, ns, kq, nf8)
    nc = _NC_CACHE[key]

    # wt[p, k*odim + o] = wr[o, k*128 + p]; fp8 k-tiles carry 1/scale
    wtf = wr.T.reshape(nk, 128, odim).copy()
    if nf8:
        wtf[nk - nf8 * kq:] /= _L2_SSCALE
    wt = np.ascontiguousarray(wtf.transpose(1, 0, 2)
                              ).astype(BF16).reshape(128, nk * odim)
    rhs_t, rhs8_t = _tile_rhs(sampled, nk, ns, ws, kq, nf8, _L2_SSCALE)

    in_maps = []
    for s in range(8):
        b, hh = s // 2, s % 2
        m = {"rhs": rhs_t[b, hh], "wt": wt}
        if nf8:
            m["rhs8"] = rhs8_t[b, hh]
        in_maps.append(m)

    kwargs = {}
    base = os.environ.get("BASS_KERNEL_TMPDIR")
    if base:
        d = os.path.join(base, f"p{os.getpid()}_call{len(DEVICE_STATS)}")
        os.makedirs(d, exist_ok=True)
        kwargs["tmpdir"] = d

    t0 = time.perf_counter_ns()
    res = bass_utils.run_bass_kernel_spmd(nc, in_maps, core_ids=list(range(8)),
                                          **kwargs)
    t1 = time.perf_counter_ns()
    DEVICE_STATS.append({"wall_ns": t1 - t0,
                         "exec_time_ns": res.exec_time_ns})
    blocks = np.stack([np.asarray(res.results[s]["out"]) for s in range(8)])
    return _untile_out(blocks, nm, ns, nch)


def _contract(sampled, wr):
    try:
        return _device_contract(sampled, wr)
    except Exception as e:  # pragma: no cover - device fallback
        import traceback
        traceback.print_exc()
        print(f"[kernel] device path failed ({e!r}); numpy fallback")
        return np.matmul(wr[None], sampled)


# ---------------------------------------------------------------- entry point
def kernel(x, w_off1, b_off1, w1, b1, g1, be1,
           w_off2, b_off2, w2, b2, g2, be2):
    x = np.asarray(x, dtype=np.float32)

    idx1, coef1 = _offsets_for_layer(x, np.asarray(w_off1), np.asarray(b_off1))
    z1 = _l1_forward(x, np.asarray(w1, dtype=np.float32))
    y1 = _gather_combine(z1, idx1, coef1)
    y1 += np.asarray(b1)[None, :, None]
    h1 = _bn_relu(y1, np.asarray(g1), np.asarray(be1)).reshape(B, MID, H, W)

    s2 = _sampled_for_layer(h1, np.asarray(w_off2), np.asarray(b_off2))
    y2 = _contract(s2, np.asarray(w2).reshape(COUT, -1))
    y2 += np.asarray(b2)[None, :, None]
    h2 = _bn_relu(y2, np.asarray(g2), np.asarray(be2)).reshape(B, COUT, H, W)
    return h2
